# revision 24
# baseline (speedup 1.0000x reference)
"""NetVLAD Trainium2 kernel v7 — chunk-paired elementwise stages.

Host ships x twice per image: xcp [C, P] fp16 (u-matmul stationary) and
xts [128px, 32t, 132] fp16 (vlad rhs; col 128 filled on-device with |x_p|),
plus one packed const tensor (cf f32 | cwo f16 | eb16 bf16 | cbi bf16).

Work unit: 1024-px chunk (8 px-tiles); elementwise softmax stages operate
on PAIRS of chunks (2048 px) to amortize per-instruction fixed costs.

  A (slot c):    [even c] ACT sq2-pair = x^2 ([C,2048])
                 PE 8 u-matmuls (psU[c]) + 8 ssq matmuls (psS[c])
                 ACT lssq(c) = ln(psS)
                 [odd c]  ACT invc-pair = exp(-.5 lssq), ncol-pair ->
                 xts[:, :, 128]
  B1 (slot c+1): DVE ls(c) = psU*invcB -> half of pair tile
  B2 (slot 2q+2): DVE negm-pair; GPS d-pair = ls+negmB;
                 ACT E-pair = exp(d + theta) -> bf16
  C  (slot 2q+3): GPS EB-pair = E*eb16; DVE scol-pair; recip; rcol;
                 GPS Ep-pair = EB[:, :, 0:56]*rcolB
  V  (2q+4/2q+5): PE psV[56,0:129] += Ep_j.T @ xts[:,j,0:129]
"""

import sys

for _p in ("/opt/trn_rl_repo",):
    if _p not in sys.path:
        sys.path.insert(0, _p)

import numpy as np

NIMG = 4
C = 128
K = 64
KE = 56
P = 4096
NCH = 4        # 1024-px chunks per image
TPC = 8        # 128-px tiles per chunk
NSLOT = NIMG * NCH
THETA = 35.0

# packed const layout (bytes per partition):
#   [0:1028)    cf  f32 [257]: cen rows0:56 cols0:128 | theta col 128 |
#               onesrow row0 cols 129:257
#   [1028:1158) cwo f16 [65]:  w^T cols 0:64 | ones col 64
#   [1158:3206) eb16 bf16 [1024]: exp(b - bmid - theta) tiled x16
#   [3206:3462) cbi bf16 [128]: identity
CPACK = 3464

_cache = {}


def _build():
    import concourse.mybir as mybir
    from concourse import bacc, tile
    from concourse.hw_specs import get_activation_tables

    f32 = mybir.dt.float32
    f16 = mybir.dt.float16
    bf16 = mybir.dt.bfloat16
    u8 = mybir.dt.uint8
    Alu = mybir.AluOpType
    Act = mybir.ActivationFunctionType

    nc = bacc.Bacc()
    xcp_in = nc.declare_dram_parameter("xcp", [NIMG, C, P], f16, isOutput=False)
    xts_in = nc.declare_dram_parameter("xts", [NIMG, C, 32, 132], f16,
                                       isOutput=False)
    cp_in = nc.declare_dram_parameter("cpack", [C, CPACK], u8, isOutput=False)
    out_ext = nc.declare_dram_parameter("out", [NIMG, KE, C], f32,
                                        isOutput=True)

    tabs = list(get_activation_tables(nc.m.arch).keys())
    tab_id = tabs.index("natural_log_exp_and_others")

    with tile.TileContext(nc) as tc:
        with (
            tc.tile_pool(name="const", bufs=1) as cpool,
            tc.tile_pool(name="xq", bufs=4) as xqpool,
            tc.tile_pool(name="xb", bufs=3) as xbpool,
            tc.tile_pool(name="xts", bufs=4) as tpool,
            tc.tile_pool(name="sq", bufs=3) as qpool,
            tc.tile_pool(name="ls", bufs=3) as lpool,
            tc.tile_pool(name="ew", bufs=3) as epool,
            tc.tile_pool(name="stats", bufs=6) as spool,
            tc.tile_pool(name="fin", bufs=2) as fpool,
            tc.tile_pool(name="psU", bufs=3, space="PSUM") as pU,
            tc.tile_pool(name="psS", bufs=2, space="PSUM") as pS,
            tc.tile_pool(name="psV", bufs=2, space="PSUM") as pV,
            tc.tile_pool(name="psT", bufs=1, space="PSUM") as pT,
        ):
            nc.scalar.add_instruction(mybir.InstLoadActFuncSet(
                name=nc.get_next_instruction_name(), ins=[], outs=[],
                act_func_set_id=tab_id))

            cpk = cpool.tile([C, CPACK], u8, tag="cpk")
            nc.sync.dma_start(cpk[:], cp_in[:])
            cf32 = cpk[:, 0:1028].bitcast(f32)
            cwo = cpk[:, 1028:1158].bitcast(f16)
            eb16 = cpk[:, 1158:3206].bitcast(bf16)
            cbi = cpk[:, 3206:3462].bitcast(bf16)

            cen = cf32[0:KE, 0:C]
            thetacol = cf32[:, 128:129]
            onesrow = cf32[0:1, 129:257]
            ident_b = cbi[:, 0:C]
            ident56_b = cbi[0:KE, 0:KE]
            wslice = cwo[:, 0:K]
            onecol = cwo[:, K:K + 1]

            # img0 x [c,p] in halves on sync ring, xts halves on gpsimd ring
            xq = []
            xts = {}
            xts[0] = tpool.tile([C, 32, 132], f16, tag="xts", name="xts0")
            for h in range(2):
                xq.append(xqpool.tile([C, P // 2], f16, tag="xq", name="xqt"))
            for p4 in range(4):
                nc.sync.dma_start(
                    xq[p4 // 2][:, (p4 % 2) * 1024:(p4 % 2 + 1) * 1024],
                    xcp_in[0, :, p4 * 1024:(p4 + 1) * 1024])
                nc.gpsimd.dma_start(xts[0][:, p4 * 8:(p4 + 1) * 8, :],
                                    xts_in[0, :, p4 * 8:(p4 + 1) * 8, :])
            xb = {}

            def fetch_xb(img):
                xb[img] = xbpool.tile([C, P], f16, tag="xb", name="xbt")
                nc.sync.dma_start(xb[img][:], xcp_in[img])

            def fetch_xts(img):
                xts[img] = tpool.tile([C, 32, 132], f16, tag="xts",
                                      name="xtst")
                nc.gpsimd.dma_start(xts[img][:], xts_in[img])

            def xchunk(c, two=False):
                img, ch = divmod(c, NCH)
                w = 2048 if two else 1024
                if img == 0:
                    h, r = divmod(ch, 2)
                    return xq[h][:, r * 1024:r * 1024 + w]
                return xb[img][:, ch * 1024:ch * 1024 + w]

            def xsrc(c, j):
                return xchunk(c)[:, j * 128:(j + 1) * 128]

            def xv(c):
                img, ch = divmod(c, NCH)
                return xts[img][:, ch * TPC:(ch + 1) * TPC, :]

            psU = {}
            psS = {}
            sq2 = {}
            lssq = {}
            invc = {}
            lsp = {}
            et = {}
            ebt = {}
            ept = {}
            scol = {}
            rcol = {}
            psV = {}
            vk = {}
            tailseq = []

            def stage_a(c):
                img, ch = divmod(c, NCH)
                q = c // 2
                depair = img == 0
                if c % 2 == 0:
                    sq2[q] = qpool.tile([C, 2048], f16, tag="sq2", name="sq2")
                    lssq[q] = spool.tile([C, 2 * TPC], f32, tag="lssq",
                                         name="lssq")
                    invc[q] = spool.tile([C, 2 * TPC], f32, tag="invc",
                                         name="invc")
                    if not depair:
                        nc.scalar.activation(sq2[q][:], xchunk(c, two=True),
                                             Act.Square)
                if depair:
                    off = (c % 2) * 1024
                    nc.scalar.activation(sq2[q][:, off:off + 1024],
                                         xchunk(c), Act.Square)
                psU[c] = pU.tile([C, TPC, K], f32, tag="psU", name="psUt")
                for j in range(TPC):
                    nc.tensor.matmul(psU[c][:, j:j + 1, :], xsrc(c, j),
                                     wslice, start=True, stop=True)
                psS[c] = pS.tile([C, TPC], f32, tag="psS", name="psSt")
                off = (c % 2) * 1024
                for j in range(TPC):
                    nc.tensor.matmul(psS[c][:, j:j + 1],
                                     sq2[q][:, off + j * 128:
                                            off + (j + 1) * 128],
                                     onecol, start=True, stop=True)
                hs = slice((c % 2) * TPC, (c % 2 + 1) * TPC)
                nc.scalar.activation(lssq[q][:, hs], psS[c][:], Act.Ln)
                if depair:
                    nc.scalar.activation(invc[q][:, hs], lssq[q][:, hs],
                                         Act.Exp, scale=-0.5)
                    ncol = xts[img][:, ch * TPC:(ch + 1) * TPC,
                                    128:129].rearrange("p t o -> p (t o)")
                    nc.scalar.activation(ncol, lssq[q][:, hs], Act.Exp,
                                         scale=0.5)
                elif c % 2 == 1:
                    nc.scalar.activation(invc[q][:], lssq[q][:], Act.Exp,
                                         scale=-0.5)
                    ncol = xts[img][:, (ch - 1) * TPC:(ch + 1) * TPC,
                                    128:129].rearrange("p t o -> p (t o)")
                    nc.scalar.activation(ncol, lssq[q][:], Act.Exp, scale=0.5)

            def stage_b1(c):
                q = c // 2
                if c % 2 == 0:
                    lsp[q] = lpool.tile([C, 2 * TPC * K], f16, tag="ls",
                                        name="ls")
                half = lsp[q][:, (c % 2) * TPC * K:(c % 2 + 1) * TPC * K]
                nc.vector.tensor_tensor(
                    half.rearrange("p (t k) -> p t k", k=K), psU[c][:, :, :],
                    invc[q][:, (c % 2) * TPC:(c % 2 + 1) * TPC]
                    .broadcast_to([C, TPC, K]), Alu.mult)

            def stage_b2(q):
                negm = spool.tile([C, 2 * TPC], f32, tag="negm", name="negm")
                nc.vector.tensor_reduce(
                    negm[:], lsp[q][:].rearrange("p (t k) -> p t k", k=K),
                    axis=mybir.AxisListType.X, op=Alu.max, negate=True)
                d = lpool.tile([C, 2 * TPC * K], f16, tag="d", name="dt")
                nc.gpsimd.tensor_tensor(
                    d[:].rearrange("p (t k) -> p t k", k=K),
                    lsp[q][:].rearrange("p (t k) -> p t k", k=K),
                    negm[:].broadcast_to([C, 2 * TPC, K]), Alu.add)
                et[q] = epool.tile([C, 2 * TPC * K], bf16, tag="E", name="et")
                nc.scalar.activation(et[q][:], d[:], Act.Exp, bias=thetacol)

            def stage_c(q):
                ebt[q] = epool.tile([C, 2 * TPC * K], bf16, tag="EB",
                                    name="ebt")
                nc.gpsimd.tensor_tensor(ebt[q][:], et[q][:], eb16, Alu.mult)
                scol[q] = spool.tile([C, 2 * TPC], f32, tag="scol",
                                     name="scol")
                nc.vector.tensor_reduce(
                    scol[q][:], ebt[q][:].rearrange("p (t k) -> p t k", k=K),
                    axis=mybir.AxisListType.X, op=Alu.add)
                gcol = spool.tile([C, 2 * TPC], f32, tag="gcol", name="gcol")
                nc.vector.reciprocal(gcol[:], scol[q][:])
                rcol[q] = spool.tile([C, 2 * TPC], f32, tag="rcol",
                                     name="rcol")
                nc.vector.tensor_tensor(rcol[q][:], invc[q][:], gcol[:],
                                        Alu.mult)
                ept[q] = epool.tile([C, 2 * TPC, KE], bf16, tag="Ep",
                                    name="ept")
                nc.gpsimd.tensor_tensor(
                    ept[q][:, :, :],
                    ebt[q][:].rearrange("p (t k) -> p t k", k=K)[:, :, 0:KE],
                    rcol[q][:].broadcast_to([C, 2 * TPC, KE]), Alu.mult)

            def vlads(c):
                img, ch = divmod(c, NCH)
                q, r = divmod(c, 2)
                if ch == 0:
                    psV[img] = pV.tile([KE, 132], f32, tag="psV", name="psVt")
                for j in range(TPC):
                    nc.tensor.matmul(psV[img][0:KE, 0:129],
                                     ept[q][:, r * TPC + j, :],
                                     xv(c)[:, j, 0:129],
                                     start=(ch == 0 and j == 0),
                                     stop=(ch == NCH - 1 and j == TPC - 1))

            def tail_a(img):
                pv = psV[img]
                negs = spool.tile([KE, 1], f32, tag="negs")
                nc.vector.tensor_scalar_mul(negs[:], pv[0:KE, 128:129], -1.0)
                vk[img] = fpool.tile([KE, C], bf16, tag="vk", name="vkt")
                nc.vector.scalar_tensor_tensor(vk[img][:], cen, negs[:],
                                               pv[0:KE, 0:C],
                                               Alu.mult, Alu.add)

            def tail_b(img):
                tt = pT.tile([C, 2, 192], f32, tag="pst", name="pst")
                t1 = tt[:, 0:1, 0:KE]
                nc.tensor.matmul(t1, vk[img][:], ident56_b,
                                 start=True, stop=True)
                tr56 = spool.tile([C, KE], bf16, tag="tr56")
                ssqk = spool.tile([C, 1], f32, tag="ssqk")
                nc.scalar.activation(tr56[:], t1, Act.Square,
                                     accum_out=ssqk[:])
                ssqc = spool.tile([C, 1], f32, tag="ssqc")
                nc.vector.tensor_scalar_max(ssqc[:], ssqk[:], 1e-24)
                lk = spool.tile([C, 1], f32, tag="lk")
                nc.scalar.activation(lk[:], ssqc[:], Act.Ln)
                invk = spool.tile([C, 1], f32, tag="invk")
                nc.scalar.activation(invk[:], lk[:], Act.Exp, scale=-0.5)
                t2 = spool.tile([C, 1], f32, tag="t2")
                nc.vector.scalar_tensor_tensor(t2[:], ssqc[:], invk[:],
                                               invk[:], Alu.mult, Alu.mult)
                tot = spool.tile([1, 1], f32, tag="tot")
                nc.gpsimd.tensor_reduce(tot[:], t2[:],
                                        axis=mybir.AxisListType.C, op=Alu.add)
                totc = spool.tile([1, 1], f32, tag="totc")
                nc.vector.tensor_scalar_max(totc[:], tot[:], 1e-24)
                ltot = spool.tile([1, 1], f32, tag="ltot")
                nc.scalar.activation(ltot[:], totc[:], Act.Ln)
                fv = spool.tile([1, 1], f32, tag="fv")
                nc.scalar.activation(fv[:], ltot[:], Act.Exp, scale=-0.5)
                nc.tensor.matmul(tt[:, 1:2, 188:189], onesrow, fv[:],
                                 start=True, stop=True)
                comb = spool.tile([C, 1], f32, tag="comb")
                nc.vector.tensor_tensor(comb[:], invk[:],
                                        tt[:, 1:2, 188:189], Alu.mult)
                vnT = fpool.tile([C, KE], bf16, tag="vnT", name="vnT")
                nc.vector.tensor_scalar(vnT[:], t1, comb[:], None, Alu.mult)
                return tt, vnT

            def tail_c(img, tt, vnT):
                nc.tensor.matmul(tt[0:KE, 1:2, 0:C], vnT[:], ident_b,
                                 start=True, stop=True)
                ob = fpool.tile([KE, C], f32, tag="ob", name="ob")
                nc.scalar.activation(ob[:], tt[0:KE, 1:2, 0:C], Act.Copy)
                nc.sync.dma_start(out_ext[img], ob[:])

            # schedule: A(sl), B1(sl-1), B2 pair at even offset, C, vlads
            for sl in range(NSLOT + 6):
                while tailseq and tailseq[0][0] <= sl:
                    tailseq.pop(0)[1]()
                # staggered input prefetch
                if sl == 0:
                    fetch_xb(1)
                if sl == 1:
                    fetch_xts(1)
                if 6 <= sl < NSLOT and sl % 4 == 2:
                    img = sl // 4 + 1
                    if img < NIMG:
                        fetch_xb(img)
                if 7 <= sl < NSLOT and sl % 4 == 3:
                    img = sl // 4 + 1
                    if img < NIMG:
                        fetch_xts(img)
                if sl < NSLOT:
                    stage_a(sl)
                if 0 <= sl - 1 < NSLOT:
                    stage_b1(sl - 1)
                b2 = sl - 2
                if 0 <= b2 < NSLOT and b2 % 2 == 1:
                    stage_b2(b2 // 2)
                c3 = sl - 3
                if 0 <= c3 < NSLOT and c3 % 2 == 1:
                    stage_c(c3 // 2)
                v = sl - 4
                if 0 <= v < NSLOT and v % 2 == 0:
                    vlads(v)
                v2 = sl - 5
                if 0 <= v2 < NSLOT and v2 % 2 == 1:
                    vlads(v2)
                    img, ch = divmod(v2, NCH)
                    if ch == NCH - 1:
                        tail_a(img)

                        def _mk(i):
                            def _b():
                                tt, vnT = tail_b(i)
                                tailseq.append(
                                    (sl + 3, lambda: tail_c(i, tt, vnT)))
                            return _b
                        tailseq.append((sl + 1, _mk(img)))
            while tailseq:
                tailseq.pop(0)[1]()

    nc.compile()
    return nc


def _get_nc():
    if "nc" not in _cache:
        _cache["nc"] = _build()
    return _cache["nc"]


def _make_in_maps(x, conv_w, conv_b, centroids):
    import ml_dtypes

    x = np.asarray(x, dtype=np.float32)
    conv_w = np.asarray(conv_w, dtype=np.float32)
    conv_b = np.asarray(conv_b, dtype=np.float32)
    centroids = np.asarray(centroids, dtype=np.float32)

    N = x.shape[0]
    n_cores = 8
    per = N // n_cores
    assert per == NIMG

    xr = x.reshape(N, C, P).astype(np.float16)
    bmid = (conv_b.max() + conv_b.min()) / 2.0
    eb = np.exp((conv_b - bmid - THETA).astype(np.float64)).astype(np.float32)

    cf = np.zeros((C, 257), dtype=np.float32)
    cf[0:KE, 0:C] = centroids[:KE]
    cf[:, 128] = THETA
    cf[0, 129:257] = 1.0
    cwo = np.zeros((C, K + 1), dtype=np.float16)
    cwo[:, 0:K] = conv_w.T.astype(np.float16)
    cwo[:, K] = 1.0
    eb16 = np.broadcast_to(np.tile(eb, 2 * TPC)[None, :],
                           (C, 2 * TPC * K)).astype(ml_dtypes.bfloat16)
    cbi = np.eye(C, dtype=np.float32).astype(ml_dtypes.bfloat16)

    cpack = np.concatenate([
        np.ascontiguousarray(cf).view(np.uint8),
        np.ascontiguousarray(cwo).view(np.uint8),
        np.ascontiguousarray(eb16).view(np.uint8),
        np.ascontiguousarray(cbi).view(np.uint8),
        np.zeros((C, 2), dtype=np.uint8),
    ], axis=1)
    assert cpack.shape == (C, CPACK), cpack.shape

    in_maps = []
    for i in range(n_cores):
        xc = np.ascontiguousarray(xr[i * per:(i + 1) * per])
        # xts[img, q, t, c] = x[img, c, t*128+q], padded to 132 cols
        xt = np.zeros((NIMG, C, 32, 132), dtype=np.float16)
        xt[:, :, :, 0:C] = xc.reshape(NIMG, C, 32, C).transpose(0, 3, 2, 1)
        in_maps.append({
            "xcp": xc,
            "xts": np.ascontiguousarray(xt),
            "cpack": cpack,
        })
    return in_maps


def kernel(x, conv_w, conv_b, centroids):
    from concourse.bass_utils import run_bass_kernel_spmd

    in_maps = _make_in_maps(x, conv_w, conv_b, centroids)
    nc = _get_nc()
    res = run_bass_kernel_spmd(nc, in_maps, list(range(8)))
    outs = [np.asarray(r["out"]).reshape(NIMG, KE * C) for r in res.results]
    return np.concatenate(outs, axis=0)


if __name__ == "__main__":
    rng = np.random.default_rng(0)
    x = rng.standard_normal((32, C, 64, 64), dtype=np.float32)
    w = rng.standard_normal((K, C), dtype=np.float32)
    b = rng.standard_normal((K,), dtype=np.float32)
    c = rng.random((K, C), dtype=np.float32)
    out = kernel(x=x, conv_w=w, conv_b=b, centroids=c)
    print(out.shape, out.dtype)


# revision 25
# speedup vs baseline: 1.0348x; 1.0348x over previous
"""NetVLAD Trainium2 kernel v6 — host-shipped transpose, PE-computed ssq,
flat 2D access patterns, packed constants, single-queue input staging.

Host ships x twice per image: xcp [C, P] fp16 (u-matmul stationary) and
xts [128px, 32t, 132] fp16 (vlad rhs; col 128 filled on-device with |x_p|),
plus one packed const tensor (cf f32 | cwo f16 | eb8 bf16 | cbi bf16).

Per 1024-px chunk (8 px-tiles), stages pipelined across slots:
  A: ACT sq2 = x^2 ([C,1024]); PE 8 u-matmuls (psU[128,8,64]) + 8 ssq
     matmuls (psS[:,j] = sq2_j^T @ ones); ACT lssq=ln(psS),
     invc=exp(-.5 lssq), ncol=exp(+.5 lssq) -> xts[:,:,128]
  B: DVE ls = psU*invcB (f16); DVE negm reduce; GPS d = ls+negmB;
     ACT E = exp(d + theta) -> bf16
  C: GPS EB = E*eb8; DVE scol reduce; DVE gcol=1/scol; DVE rcol;
     GPS Ep = EB[:, :, 0:56]*rcolB
  V (lag 3): PE psV[56,0:129] += Ep_j.T @ xts[:,j,0:129]
"""

import sys

for _p in ("/opt/trn_rl_repo",):
    if _p not in sys.path:
        sys.path.insert(0, _p)

import numpy as np

NIMG = 4
C = 128
K = 64
KE = 56
P = 4096
NCH = 4        # 1024-px chunks per image
TPC = 8        # 128-px tiles per chunk
NSLOT = NIMG * NCH
LAG = 3
THETA = 35.0

# packed const layout (bytes per partition):
#   [0:1028)    cf  f32 [257]: cen rows0:56 cols0:128 | theta col 128 |
#               onesrow row0 cols 129:257
#   [1028:1158) cwo f16 [65]:  w^T cols 0:64 | ones col 64
#   [1158:2182) eb8 bf16 [512]: exp(b - bmid - theta) tiled x8
#   [2182:2438) cbi bf16 [128]: identity
CPACK = 2440

_cache = {}


def _build():
    import concourse.mybir as mybir
    from concourse import bacc, tile
    from concourse.hw_specs import get_activation_tables

    f32 = mybir.dt.float32
    f16 = mybir.dt.float16
    bf16 = mybir.dt.bfloat16
    u8 = mybir.dt.uint8
    Alu = mybir.AluOpType
    Act = mybir.ActivationFunctionType

    nc = bacc.Bacc()
    xcp_in = nc.declare_dram_parameter("xcp", [NIMG, C, P], f16, isOutput=False)
    xts_in = nc.declare_dram_parameter("xts", [NIMG, C, 32, 132], f16,
                                       isOutput=False)
    cp_in = nc.declare_dram_parameter("cpack", [C, CPACK], u8, isOutput=False)
    out_ext = nc.declare_dram_parameter("out", [NIMG, KE, C], f32,
                                        isOutput=True)

    tabs = list(get_activation_tables(nc.m.arch).keys())
    tab_id = tabs.index("natural_log_exp_and_others")

    with tile.TileContext(nc) as tc:
        with (
            tc.tile_pool(name="const", bufs=1) as cpool,
            tc.tile_pool(name="xq", bufs=4) as xqpool,
            tc.tile_pool(name="xb", bufs=3) as xbpool,
            tc.tile_pool(name="xts", bufs=4) as tpool,
            tc.tile_pool(name="sq", bufs=3) as qpool,
            tc.tile_pool(name="ls", bufs=3) as lpool,
            tc.tile_pool(name="ew", bufs=3) as epool,
            tc.tile_pool(name="stats", bufs=6) as spool,
            tc.tile_pool(name="fin", bufs=2) as fpool,
            tc.tile_pool(name="psU", bufs=3, space="PSUM") as pU,
            tc.tile_pool(name="psS", bufs=2, space="PSUM") as pS,
            tc.tile_pool(name="psV", bufs=2, space="PSUM") as pV,
            tc.tile_pool(name="psT", bufs=1, space="PSUM") as pT,
        ):
            nc.scalar.add_instruction(mybir.InstLoadActFuncSet(
                name=nc.get_next_instruction_name(), ins=[], outs=[],
                act_func_set_id=tab_id))

            cpk = cpool.tile([C, CPACK], u8, tag="cpk")
            nc.sync.dma_start(cpk[:], cp_in[:])
            cf32 = cpk[:, 0:1028].bitcast(f32)
            cwo = cpk[:, 1028:1158].bitcast(f16)
            eb8 = cpk[:, 1158:2182].bitcast(bf16)
            cbi = cpk[:, 2182:2438].bitcast(bf16)

            cen = cf32[0:KE, 0:C]
            thetacol = cf32[:, 128:129]
            onesrow = cf32[0:1, 129:257]
            ident_b = cbi[:, 0:C]
            ident56_b = cbi[0:KE, 0:KE]
            wslice = cwo[:, 0:K]
            onecol = cwo[:, K:K + 1]

            # x [c,p] for u-matmul stationary; img0 in halves on sync ring,
            # xts (transposed) on gpsimd ring
            xq = []
            xts = {}
            xts[0] = tpool.tile([C, 32, 132], f16, tag="xts", name="xts0")
            for p4 in range(4):
                t = xqpool.tile([C, P // 4], f16, tag="xq", name="xqt")
                nc.sync.dma_start(t[:], xcp_in[0, :, p4 * 1024:(p4 + 1) * 1024])
                xq.append(t)
                nc.gpsimd.dma_start(xts[0][:, p4 * 8:(p4 + 1) * 8, :],
                                    xts_in[0, :, p4 * 8:(p4 + 1) * 8, :])
            xb = {}

            def fetch_xb(img):
                xb[img] = xbpool.tile([C, P], f16, tag="xb", name="xbt")
                nc.sync.dma_start(xb[img][:], xcp_in[img])

            def fetch_xts(img):
                xts[img] = tpool.tile([C, 32, 132], f16, tag="xts",
                                      name="xtst")
                nc.gpsimd.dma_start(xts[img][:], xts_in[img])

            def xchunk(c):
                img, ch = divmod(c, NCH)
                if img == 0:
                    return xq[ch][:]
                return xb[img][:, ch * 1024:(ch + 1) * 1024]

            def xsrc(c, j):
                return xchunk(c)[:, j * 128:(j + 1) * 128]

            def xv(c):
                img, ch = divmod(c, NCH)
                return xts[img][:, ch * TPC:(ch + 1) * TPC, :]

            psU = {}
            psS = {}
            st = {}
            et = {}
            ebt = {}
            ept = {}
            psV = {}
            vk = {}
            tailseq = []

            def stage_a(c):
                img, ch = divmod(c, NCH)
                if img + 1 < NIMG:
                    if ch == 0 and img + 1 not in xb:
                        fetch_xb(img + 1)
                    if ch == 1 and img + 1 not in xts:
                        fetch_xts(img + 1)
                sq2 = qpool.tile([C, 1024], f16, tag="sq2", name="sq2")
                nc.scalar.activation(sq2[:], xchunk(c), Act.Square)
                psU[c] = pU.tile([C, TPC, K], f32, tag="psU", name="psUt")
                for j in range(TPC):
                    nc.tensor.matmul(psU[c][:, j:j + 1, :], xsrc(c, j),
                                     wslice, start=True, stop=True)
                psS[c] = pS.tile([C, TPC], f32, tag="psS", name="psSt")
                for j in range(TPC):
                    nc.tensor.matmul(psS[c][:, j:j + 1],
                                     sq2[:, j * 128:(j + 1) * 128],
                                     onecol, start=True, stop=True)
                s = {}
                s["lssq"] = spool.tile([C, TPC], f32, tag="lssq", name="lssq")
                nc.scalar.activation(s["lssq"][:], psS[c][:], Act.Ln)
                s["invc"] = spool.tile([C, TPC], f32, tag="invc", name="invc")
                nc.scalar.activation(s["invc"][:], s["lssq"][:], Act.Exp,
                                     scale=-0.5)
                ncol = xv(c)[:, :, 128:129].rearrange("p t o -> p (t o)")
                nc.scalar.activation(ncol, s["lssq"][:], Act.Exp, scale=0.5)
                st[c] = s

            def stage_b(c):
                s = st[c]
                ls = lpool.tile([C, TPC * K], f16, tag="ls", name="ls")
                nc.vector.tensor_tensor(
                    ls[:].rearrange("p (t k) -> p t k", k=K), psU[c][:, :, :],
                    s["invc"][:].broadcast_to([C, TPC, K]), Alu.mult)
                s["negm"] = spool.tile([C, TPC], f32, tag="negm", name="negm")
                nc.vector.tensor_reduce(
                    s["negm"][:], ls[:].rearrange("p (t k) -> p t k", k=K),
                    axis=mybir.AxisListType.X, op=Alu.max, negate=True)
                d = lpool.tile([C, TPC * K], f16, tag="d", name="dt")
                nc.gpsimd.tensor_tensor(
                    d[:].rearrange("p (t k) -> p t k", k=K),
                    ls[:].rearrange("p (t k) -> p t k", k=K),
                    s["negm"][:].broadcast_to([C, TPC, K]), Alu.add)
                et[c] = epool.tile([C, TPC * K], bf16, tag="E", name="et")
                nc.scalar.activation(et[c][:], d[:], Act.Exp, bias=thetacol)

            def stage_c(c):
                s = st[c]
                ebt[c] = epool.tile([C, TPC * K], bf16, tag="EB", name="ebt")
                nc.gpsimd.tensor_tensor(ebt[c][:], et[c][:], eb8, Alu.mult)
                s["scol"] = spool.tile([C, TPC], f32, tag="scol", name="scol")
                nc.vector.tensor_reduce(
                    s["scol"][:], ebt[c][:].rearrange("p (t k) -> p t k", k=K),
                    axis=mybir.AxisListType.X, op=Alu.add)
                s["gcol"] = spool.tile([C, TPC], f32, tag="gcol", name="gcol")
                nc.vector.reciprocal(s["gcol"][:], s["scol"][:])
                s["rcol"] = spool.tile([C, TPC], f32, tag="rcol", name="rcol")
                nc.vector.tensor_tensor(s["rcol"][:], s["invc"][:],
                                        s["gcol"][:], Alu.mult)
                ept[c] = epool.tile([C, TPC, KE], bf16, tag="Ep", name="ept")
                nc.gpsimd.tensor_tensor(
                    ept[c][:, :, :],
                    ebt[c][:].rearrange("p (t k) -> p t k", k=K)[:, :, 0:KE],
                    s["rcol"][:].broadcast_to([C, TPC, KE]), Alu.mult)

            def vlads(c):
                img, ch = divmod(c, NCH)
                if ch == 0:
                    psV[img] = pV.tile([KE, 132], f32, tag="psV", name="psVt")
                for j in range(TPC):
                    nc.tensor.matmul(psV[img][0:KE, 0:129],
                                     ept[c][:, j, :],
                                     xv(c)[:, j, 0:129],
                                     start=(ch == 0 and j == 0),
                                     stop=(ch == NCH - 1 and j == TPC - 1))

            def tail_a(img):
                pv = psV[img]
                negs = spool.tile([KE, 1], f32, tag="negs")
                nc.vector.tensor_scalar_mul(negs[:], pv[0:KE, 128:129], -1.0)
                vk[img] = fpool.tile([KE, C], bf16, tag="vk", name="vkt")
                nc.vector.scalar_tensor_tensor(vk[img][:], cen, negs[:],
                                               pv[0:KE, 0:C],
                                               Alu.mult, Alu.add)

            def tail_b(img):
                tt = pT.tile([C, 2, 192], f32, tag="pst", name="pst")
                t1 = tt[:, 0:1, 0:KE]
                nc.tensor.matmul(t1, vk[img][:], ident56_b,
                                 start=True, stop=True)
                tr56 = spool.tile([C, KE], bf16, tag="tr56")
                ssqk = spool.tile([C, 1], f32, tag="ssqk")
                nc.scalar.activation(tr56[:], t1, Act.Square,
                                     accum_out=ssqk[:])
                ssqc = spool.tile([C, 1], f32, tag="ssqc")
                nc.vector.tensor_scalar_max(ssqc[:], ssqk[:], 1e-24)
                lk = spool.tile([C, 1], f32, tag="lk")
                nc.scalar.activation(lk[:], ssqc[:], Act.Ln)
                invk = spool.tile([C, 1], f32, tag="invk")
                nc.scalar.activation(invk[:], lk[:], Act.Exp, scale=-0.5)
                t2 = spool.tile([C, 1], f32, tag="t2")
                nc.vector.scalar_tensor_tensor(t2[:], ssqc[:], invk[:],
                                               invk[:], Alu.mult, Alu.mult)
                tot = spool.tile([1, 1], f32, tag="tot")
                nc.gpsimd.tensor_reduce(tot[:], t2[:],
                                        axis=mybir.AxisListType.C, op=Alu.add)
                totc = spool.tile([1, 1], f32, tag="totc")
                nc.vector.tensor_scalar_max(totc[:], tot[:], 1e-24)
                ltot = spool.tile([1, 1], f32, tag="ltot")
                nc.scalar.activation(ltot[:], totc[:], Act.Ln)
                fv = spool.tile([1, 1], f32, tag="fv")
                nc.scalar.activation(fv[:], ltot[:], Act.Exp, scale=-0.5)
                nc.tensor.matmul(tt[:, 1:2, 188:189], onesrow, fv[:],
                                 start=True, stop=True)
                comb = spool.tile([C, 1], f32, tag="comb")
                nc.vector.tensor_tensor(comb[:], invk[:],
                                        tt[:, 1:2, 188:189], Alu.mult)
                vnT = fpool.tile([C, KE], bf16, tag="vnT", name="vnT")
                nc.vector.tensor_scalar(vnT[:], t1, comb[:], None, Alu.mult)
                return tt, vnT

            def tail_c(img, tt, vnT):
                nc.tensor.matmul(tt[0:KE, 1:2, 0:C], vnT[:], ident_b,
                                 start=True, stop=True)
                ob = fpool.tile([KE, C], f32, tag="ob", name="ob")
                nc.scalar.activation(ob[:], tt[0:KE, 1:2, 0:C], Act.Copy)
                nc.sync.dma_start(out_ext[img], ob[:])

            for sl in range(NSLOT + LAG + 2):
                while tailseq and tailseq[0][0] <= sl:
                    tailseq.pop(0)[1]()
                if sl < NSLOT:
                    stage_a(sl)
                v = sl - LAG
                if 0 <= v < NSLOT:
                    vlads(v)
                    img, ch = divmod(v, NCH)
                    if ch == NCH - 1:
                        tail_a(img)

                        def _mk(i):
                            def _b():
                                tt, vnT = tail_b(i)
                                tailseq.append(
                                    (sl + 3, lambda: tail_c(i, tt, vnT)))
                            return _b
                        tailseq.append((sl + 1, _mk(img)))
                if 0 <= sl - 1 < NSLOT:
                    stage_b(sl - 1)
                if 0 <= sl - 2 < NSLOT:
                    stage_c(sl - 2)
            while tailseq:
                tailseq.pop(0)[1]()

    nc.compile()
    return nc


def _get_nc():
    if "nc" not in _cache:
        _cache["nc"] = _build()
    return _cache["nc"]


def _make_in_maps(x, conv_w, conv_b, centroids):
    import ml_dtypes

    x = np.asarray(x, dtype=np.float32)
    conv_w = np.asarray(conv_w, dtype=np.float32)
    conv_b = np.asarray(conv_b, dtype=np.float32)
    centroids = np.asarray(centroids, dtype=np.float32)

    N = x.shape[0]
    n_cores = 8
    per = N // n_cores
    assert per == NIMG

    xr = x.reshape(N, C, P).astype(np.float16)
    bmid = (conv_b.max() + conv_b.min()) / 2.0
    eb = np.exp((conv_b - bmid - THETA).astype(np.float64)).astype(np.float32)

    cf = np.zeros((C, 257), dtype=np.float32)
    cf[0:KE, 0:C] = centroids[:KE]
    cf[:, 128] = THETA
    cf[0, 129:257] = 1.0
    cwo = np.zeros((C, K + 1), dtype=np.float16)
    cwo[:, 0:K] = conv_w.T.astype(np.float16)
    cwo[:, K] = 1.0
    eb8 = np.broadcast_to(np.tile(eb, TPC)[None, :], (C, TPC * K)).astype(
        ml_dtypes.bfloat16)
    cbi = np.eye(C, dtype=np.float32).astype(ml_dtypes.bfloat16)

    cpack = np.concatenate([
        np.ascontiguousarray(cf).view(np.uint8),
        np.ascontiguousarray(cwo).view(np.uint8),
        np.ascontiguousarray(eb8).view(np.uint8),
        np.ascontiguousarray(cbi).view(np.uint8),
        np.zeros((C, 2), dtype=np.uint8),
    ], axis=1)
    assert cpack.shape == (C, CPACK), cpack.shape

    in_maps = []
    for i in range(n_cores):
        xc = np.ascontiguousarray(xr[i * per:(i + 1) * per])
        # xts[img, q, t, c] = x[img, c, t*128+q], padded to 132 cols
        xt = np.zeros((NIMG, C, 32, 132), dtype=np.float16)
        xt[:, :, :, 0:C] = xc.reshape(NIMG, C, 32, C).transpose(0, 3, 2, 1)
        in_maps.append({
            "xcp": xc,
            "xts": np.ascontiguousarray(xt),
            "cpack": cpack,
        })
    return in_maps


def kernel(x, conv_w, conv_b, centroids):
    from concourse.bass_utils import run_bass_kernel_spmd

    in_maps = _make_in_maps(x, conv_w, conv_b, centroids)
    nc = _get_nc()
    res = run_bass_kernel_spmd(nc, in_maps, list(range(8)))
    outs = [np.asarray(r["out"]).reshape(NIMG, KE * C) for r in res.results]
    return np.concatenate(outs, axis=0)


if __name__ == "__main__":
    rng = np.random.default_rng(0)
    x = rng.standard_normal((32, C, 64, 64), dtype=np.float32)
    w = rng.standard_normal((K, C), dtype=np.float32)
    b = rng.standard_normal((K,), dtype=np.float32)
    c = rng.random((K, C), dtype=np.float32)
    out = kernel(x=x, conv_w=w, conv_b=b, centroids=c)
    print(out.shape, out.dtype)
